# revision 15
# baseline (speedup 1.0000x reference)
"""CRF loss (forward algorithm + gold score) on 8 trn2 NeuronCores.

Data-parallel over batch (32 sequences/core). The forward recurrence runs in
probability space with bf16 matmul operands (fp32 PSUM accumulation):
    v_t = (E^T @ v_{t-1}) * exp(e_t - c0),   E = exp(transitions)

Serial-depth reduction: each sequence's 511 steps are split into K=23
segments processed CONCURRENTLY as matmul columns. Segment k starts at
t=k*L from the pseudo-init exp(e_{kL} - cf); after W=5 warmup steps the
chain direction has converged (random positive matrices contract direction
error ~10x/step), so log-colsum snapshots telescope exactly:
    logZ_b = sum_k ln(colsum_k @ slot N) - sum_{k>=1} ln(colsum_k @ slot W)
             + cf + 511*c0
This needs only N = L+W = 27 serial slots of [128, 736] tiles instead of
511 serial steps of [128, 32] — amortizing per-instruction overhead, which
dominated the previous version.

Gold-path score runs on GPSIMD (Pool) gathers + reduces, off the chain's
critical path:
  - emissions: fp8 per-sequence quarter-chunk layout, wrapped shared-index
    indirect_copy, compile-time 0/1 mask selects the owning partition.
  - transitions: fp8 host-replicated [128, T*T] table, host-built uint16
    pair indices.
Gold quantization (fp8 e4m3) adds ~0.009 abs error on a ~2991 loss.

Each core returns partial sums; the host combines them into the scalar loss.
"""

import numpy as np
import ml_dtypes

import concourse.bacc as bacc
import concourse.mybir as mybir
import concourse.tile as tile
from concourse.bass_utils import run_bass_kernel_spmd
from concourse.mybir import AluOpType
from concourse.bass_isa import ReduceOp

F32 = mybir.dt.float32
F16 = mybir.dt.float16
BF16 = mybir.dt.bfloat16
FP8 = mybir.dt.float8e4
I32 = mybir.dt.int32
U16 = mybir.dt.uint16

B, S, T = 256, 512, 128
NCORES = 8
BL = B // NCORES          # 32 sequences per core
K = 23                    # segments per sequence
W = 5                     # warmup steps per segment
L = (S - 1 - W) // K      # 22 payload steps per segment
N = L + W                 # 27 serial slots
COLS = K * BL             # 736 chain columns per core
HALF = COLS // 2          # 368 columns per matmul group
CH = 4                    # slots per emission chunk
NCH = (N + 1 + CH - 1) // CH  # 7 chunks covering tau = 0..27

# log-domain normalization constants (E[log colsum] of this recurrence;
# anything within a few nats works — fp32 has ~88 nats of range)
C0 = 5.843
C_FIRST = 5.337

NGS = BL // 8             # 4 gather sets (8 seqs each)
QS = S // 16              # 32 steps per partition-quarter

ACT_EXP = mybir.ActivationFunctionType.Exp
ACT_LN = mybir.ActivationFunctionType.Ln


def build_nc():
    """Build the SPMD single-core program (identical on all cores)."""
    nc = bacc.Bacc("TRN2", target_bir_lowering=False, debug=False,
                   enable_asserts=False)

    ems = nc.dram_tensor("ems", [T, (N + 1) * COLS], F16,
                         kind="ExternalInput").ap()
    emq8 = nc.dram_tensor("emq8", [NGS, 128, QS * T], FP8,
                          kind="ExternalInput").ap()
    eidxw = nc.dram_tensor("eidxw", [NGS, 128, S // 16], U16,
                           kind="ExternalInput").ap()
    tridxw = nc.dram_tensor("tridxw", [NGS, 128, S // 16], U16,
                            kind="ExternalInput").ap()
    ttab8 = nc.dram_tensor("ttab8", [128, T * T], FP8,
                           kind="ExternalInput").ap()
    maskq = nc.dram_tensor("maskq", [128, S], F32, kind="ExternalInput").ap()
    trans = nc.dram_tensor("trans", [T, T], F32, kind="ExternalInput").ap()
    out = nc.dram_tensor("out", [1, 16], F32, kind="ExternalOutput").ap()

    with tile.TileContext(nc) as tc:
        with (
            tc.tile_pool(name="const", bufs=1) as cpool,
            tc.tile_pool(name="raw", bufs=3) as rawpool,
            tc.tile_pool(name="fch", bufs=3) as fpool,
            tc.tile_pool(name="va", bufs=3) as vapool,
            tc.tile_pool(name="vb", bufs=3) as vbpool,
            tc.tile_pool(name="psa", bufs=3, space="PSUM") as psapool,
            tc.tile_pool(name="psb", bufs=3, space="PSUM") as psbpool,
            tc.tile_pool(name="pscs", bufs=2, space="PSUM") as cspool,
            tc.tile_pool(name="gold", bufs=1) as gpool,
            tc.tile_pool(name="eq", bufs=2) as eqpool,
            tc.tile_pool(name="gout", bufs=2) as gopool,
        ):
            # ---- stationary weights: E = exp(trans) ----
            tr_raw = cpool.tile([T, T], F32)
            nc.sync.dma_start(tr_raw[:], trans)
            E = cpool.tile([T, T], BF16)
            nc.scalar.activation(E[:], tr_raw[:], ACT_EXP)

            bias_c0 = cpool.tile([128, 1], F32)
            nc.vector.memset(bias_c0[:], -C0)
            bias_cf = cpool.tile([128, 1], F32)
            nc.vector.memset(bias_cf[:], -C_FIRST)
            ones = cpool.tile([T, 1], BF16)
            nc.vector.memset(ones[:], 1.0)

            chunk_f = {}

            def load_chunk(c):
                raw = rawpool.tile([T, CH * COLS], F16)
                nc.sync.dma_start(raw[:],
                                  ems[:, c * CH * COLS:(c + 1) * CH * COLS])
                fc = fpool.tile([T, CH * COLS], BF16)
                nc.scalar.activation(fc[:], raw[:], ACT_EXP, bias=bias_c0[:])
                chunk_f[c] = fc
                return raw, fc

            # chunk 0 covers tau=0..3; init v from tau=0 columns.
            # F-exp split per tau so slot 1 isn't gated on the whole chunk.
            raw0 = rawpool.tile([T, CH * COLS], F16)
            nc.sync.dma_start(raw0[:], ems[:, 0:CH * COLS])
            vA = vapool.tile([T, HALF], BF16)
            nc.scalar.activation(vA[:], raw0[:, 0:HALF], ACT_EXP,
                                 bias=bias_cf[:])
            vB = vbpool.tile([T, HALF], BF16)
            nc.scalar.activation(vB[:], raw0[:, HALF:COLS], ACT_EXP,
                                 bias=bias_cf[:])
            f_cur = fpool.tile([T, CH * COLS], BF16)
            for t in range(1, CH):
                nc.scalar.activation(f_cur[:, t * COLS:(t + 1) * COLS],
                                     raw0[:, t * COLS:(t + 1) * COLS],
                                     ACT_EXP, bias=bias_c0[:])
            chunk_f[0] = f_cur

            # ---- gold score tiles ----
            mask_t = gpool.tile([128, S], F32)
            ttab_t = gpool.tile([128, T * T], FP8)
            ecols = gpool.tile([128, NGS], F32)
            tcols = gpool.tile([128, NGS], F32)
            eidx_t = []
            tridx_t = []
            for g in range(NGS):
                ei = gpool.tile([128, S // 16], U16, tag=f"eidx{g}",
                                name=f"eidx{g}")
                eidx_t.append(ei)
                ti = gpool.tile([128, S // 16], U16, tag=f"tridx{g}",
                                name=f"tridx{g}")
                tridx_t.append(ti)

            gold_ops = {}

            def at(slot, fn):
                gold_ops.setdefault(slot, []).append(fn)

            def _prep():
                nc.gpsimd.dma_start(mask_t[:], maskq)
                for g in range(NGS):
                    nc.gpsimd.dma_start(eidx_t[g][:], eidxw[g])
                    nc.gpsimd.dma_start(tridx_t[g][:], tridxw[g])
            at(1, _prep)

            # transitions table: 4 sub-DMAs spread over early slots
            qt = T * T // 4
            for i in range(4):
                def _tdma(i=i):
                    nc.gpsimd.dma_start(ttab_t[:, i * qt:(i + 1) * qt],
                                        ttab8[:, i * qt:(i + 1) * qt])
                at(1 + i, _tdma)

            eq_t = [None] * NGS

            def make_set(g):
                def _eqdma():
                    eq_t[g] = eqpool.tile([128, QS * T], FP8, tag="eq", name=f"eq{g}")
                    nc.gpsimd.dma_start(eq_t[g][:], emq8[g])

                geo = gopool.tile([128, S], FP8, tag="geo")
                gem = gopool.tile([128, S], F32, tag="gem")
                gems = gopool.tile([128, S], F32, tag="gems")

                def _egather():
                    nc.gpsimd.indirect_copy(
                        geo[:], eq_t[g][:], eidx_t[g][:],
                        i_know_ap_gather_is_preferred=True)

                def _ereduce():
                    # (geo * mask) on Pool (fp8-safe, software), then
                    # free-axis sum via ACT copy-accumulate
                    nc.gpsimd.tensor_tensor(gem[:], geo[:], mask_t[:],
                                            AluOpType.mult)
                    nc.scalar.activation(gems[:], gem[:],
                                         mybir.ActivationFunctionType.Copy,
                                         accum_out=ecols[:, g:g + 1])

                gto = gopool.tile([128, S - 1], FP8, tag="gto")
                gtos = gopool.tile([128, S - 1], F32, tag="gtos")

                def _tgather():
                    nc.gpsimd.indirect_copy(
                        gto[:], ttab_t[:], tridx_t[g][:],
                        i_know_ap_gather_is_preferred=True)

                def _treduce():
                    # free-axis sum via ACT copy-accumulate (scalar engine)
                    nc.scalar.activation(gtos[:], gto[:],
                                         mybir.ActivationFunctionType.Copy,
                                         accum_out=tcols[:, g:g + 1])

                at(2 + 5 * g, _eqdma)
                at(4 + 5 * g, _egather)
                at(5 + 5 * g, _ereduce)
                at(8 + 4 * g, _tgather)
                at(9 + 4 * g, _treduce)

            for g in range(NGS):
                make_set(g)


            def _final_gold():
                eall = gpool.tile([128, NGS], F32, name="eall")
                nc.gpsimd.partition_all_reduce(eall[:], ecols[:], channels=128,
                                               reduce_op=ReduceOp.add)
                tall = gpool.tile([128, NGS], F32, name="tall")
                nc.gpsimd.partition_all_reduce(tall[:], tcols[:], channels=128,
                                               reduce_op=ReduceOp.add)
                _final_gold.eall = eall
                _final_gold.tall = tall
            at(25, _final_gold)

            # ln-accumulator tiles
            ln_s = gpool.tile([1, HALF], F32)       # scratch for Ln output

            l0a = gpool.tile([1, 1], F32)
            l0b = gpool.tile([1, 1], F32)
            l1a = gpool.tile([1, 1], F32)
            l1b = gpool.tile([1, 1], F32)

            # ---- the chain: 27 slots, 2 column groups each ----
            for tau in range(1, N + 1):
                # prefetch: chunk c covers tau in [4c, 4c+3]; issue 3 ahead
                if tau % CH == 1 and (tau + 3) // CH < NCH:
                    load_chunk((tau + 3) // CH)
                if tau % CH == 0:
                    f_cur = chunk_f[tau // CH]
                off = (tau % CH) * COLS

                psA = psapool.tile([T, HALF], F32)
                nc.tensor.matmul(psA[:], lhsT=E[:], rhs=vA[:],
                                 start=True, stop=True)
                vA = vapool.tile([T, HALF], BF16)
                nc.vector.tensor_tensor(vA[:], psA[:],
                                        f_cur[:, off:off + HALF],
                                        AluOpType.mult)

                psB = psbpool.tile([T, HALF], F32)
                nc.tensor.matmul(psB[:], lhsT=E[:], rhs=vB[:],
                                 start=True, stop=True)
                vB = vbpool.tile([T, HALF], BF16)
                nc.vector.tensor_tensor(vB[:], psB[:],
                                        f_cur[:, off + HALF:off + COLS],
                                        AluOpType.mult)

                if tau == W:
                    # boundary snapshot: ln colsums, excluding segment 0
                    # (columns 0..BL-1 of group A)
                    cs0a = cspool.tile([1, HALF], F32, tag="cs", name="cs0a")
                    nc.tensor.matmul(cs0a[:, 0:HALF - BL], lhsT=ones[:],
                                     rhs=vA[:, BL:HALF], start=True, stop=True)
                    nc.scalar.activation(ln_s[:, 0:HALF - BL],
                                         cs0a[:, 0:HALF - BL], ACT_LN,
                                         accum_out=l0a[:])
                    cs0b = cspool.tile([1, HALF], F32, tag="cs", name="cs0b")
                    nc.tensor.matmul(cs0b[:], lhsT=ones[:], rhs=vB[:],
                                     start=True, stop=True)
                    nc.scalar.activation(ln_s[:], cs0b[:], ACT_LN,
                                         accum_out=l0b[:])

                for fn in gold_ops.get(tau, []):
                    fn()

            # ---- tail: final colsums and ln-accumulate ----
            cs1a = cspool.tile([1, HALF], F32, tag="cs", name="cs1a")
            nc.tensor.matmul(cs1a[:], lhsT=ones[:], rhs=vA[:],
                             start=True, stop=True)
            nc.scalar.activation(ln_s[:], cs1a[:], ACT_LN, accum_out=l1a[:])
            cs1b = cspool.tile([1, HALF], F32, tag="cs", name="cs1b")
            nc.tensor.matmul(cs1b[:], lhsT=ones[:], rhs=vB[:],
                             start=True, stop=True)
            nc.scalar.activation(ln_s[:], cs1b[:], ACT_LN, accum_out=l1b[:])

            # ---- assemble output ----
            osb = gpool.tile([1, 16], F32)
            nc.vector.memset(osb[:], 0.0)
            nc.vector.tensor_copy(osb[:, 0:1], l1a[:])
            nc.vector.tensor_copy(osb[:, 1:2], l1b[:])
            nc.vector.tensor_copy(osb[:, 2:3], l0a[:])
            nc.vector.tensor_copy(osb[:, 3:4], l0b[:])
            nc.vector.tensor_copy(osb[:, 4:4 + NGS], _final_gold.eall[0:1, :])
            nc.vector.tensor_copy(osb[:, 8:8 + NGS], _final_gold.tall[0:1, :])
            nc.sync.dma_start(out, osb[:])

    nc.compile()
    return nc


_NC_CACHE = {}


def _get_nc(key=0):
    if key not in _NC_CACHE:
        _NC_CACHE[key] = build_nc()
    return _NC_CACHE[key]


def make_in_maps(emissions, tags, transitions):
    """Shard full inputs into per-core input maps (host-side, layout only)."""
    emissions = np.ascontiguousarray(emissions, dtype=np.float32)
    transitions = np.ascontiguousarray(transitions, dtype=np.float32)
    tags = np.asarray(tags).astype(np.int32)

    # mask[p, s] = 1 where partition p's quarter-chunk holds step s
    pp = np.arange(128) % 16
    ss = np.arange(S) // QS
    maskq = np.ascontiguousarray((pp[:, None] == ss[None, :]).astype(np.float32))

    ttab8 = np.ascontiguousarray(
        np.broadcast_to(
            transitions.reshape(1, T * T).astype(ml_dtypes.float8_e4m3),
            (128, T * T)))

    in_maps = []
    for c in range(NCORES):
        em_c = emissions[c * BL:(c + 1) * BL]               # [bl, S, T]
        tg = tags[c * BL:(c + 1) * BL]                      # [bl, S]

        # chain stream: ems[p, tau, k*BL + b] = em_c[b, k*L + tau, p]
        A = np.empty((T, N + 1, K, BL), dtype=np.float16)
        for k in range(K):
            A[:, :, k, :] = em_c[:, k * L:k * L + N + 1, :].transpose(2, 1, 0)
        ems = np.ascontiguousarray(A.reshape(T, (N + 1) * COLS))

        emq8 = np.ascontiguousarray(
            em_c.reshape(NGS, 128, QS * T).astype(ml_dtypes.float8_e4m3))

        # wrapped shared gather indices: out position i of group j reads
        # idx[16j + i%16, i//16]
        eidxw = np.zeros((NGS, 128, S // 16), dtype=np.uint16)
        tridxw = np.zeros((NGS, 128, S // 16), dtype=np.uint16)
        i_arr = np.arange(S)
        for g in range(NGS):
            for j in range(8):
                b = 8 * g + j
                # emissions: index (s % QS)*T + tag[b, s], s = i
                ei = ((i_arr % QS) * T + tg[b, i_arr]).astype(np.uint16)
                eidxw[g, 16 * j + (i_arr % 16), i_arr // 16] = ei
                # transitions: tag[b, s]*T + tag[b, s+1], s = i < S-1
                i2 = i_arr[:S - 1]
                ti = (tg[b, i2] * T + tg[b, i2 + 1]).astype(np.uint16)
                tridxw[g, 16 * j + (i2 % 16), i2 // 16] = ti

        in_maps.append({"ems": ems, "emq8": emq8,
                        "eidxw": np.ascontiguousarray(eidxw),
                        "tridxw": np.ascontiguousarray(tridxw),
                        "ttab8": ttab8, "maskq": maskq,
                        "trans": transitions})
    return in_maps


def combine(outs):
    """Unshard: combine per-core partial sums into the scalar loss."""
    logz_sum = sum(float(o[0, 0]) + float(o[0, 1])
                   - float(o[0, 2]) - float(o[0, 3]) for o in outs)
    emit_sum = sum(float(o[0, 4 + g]) for o in outs for g in range(NGS))
    trans_sum = sum(float(o[0, 8 + g]) for o in outs for g in range(NGS)) / 16.0
    logz_mean = logz_sum / B + C_FIRST + (S - 1) * C0
    gold_mean = (emit_sum + trans_sum) / B
    return np.float32(logz_mean - gold_mean)


def kernel(emissions, tags, transitions):
    nc = _get_nc()
    in_maps = make_in_maps(emissions, tags, transitions)
    res = run_bass_kernel_spmd(nc, in_maps, core_ids=list(range(NCORES)))
    outs = [r["out"] for r in res.results]
    return combine(outs)
